# revision 1
# baseline (speedup 1.0000x reference)
"""Attention pooling kernel for Trainium2 (8 NeuronCores, data-parallel).

Computes, per example b:
    energy[s] = tanh(dot(x[b, s, :], w))
    attn      = softmax(energy) over s
    out[b, h] = sum_s attn[s] * x[b, s, h]

tanh bounds energy to [-1, 1], so exp() needs no max-subtraction: we
accumulate the unnormalized weighted sum and the denominator in one pass
over the data (single HBM read of x — the memory roofline).

Per-core mapping (shard = B/8 = 4 examples):
  - DMA: stream x in supertiles [128 rows, CH*1024]; rows are mapped
    p-major (s = p*CH + c) so each partition reads one contiguous
    CH*4KB run from DRAM. The softmax sums are permutation-invariant
    across rows, so any row->$(p,c) mapping works.
  - VectorE: elementwise x*w product
  - ScalarE: Copy-activation with accum_out (free-dim sum) -> energy per row,
    then tanh and exp (the fused DVE reduce ops fail on this runtime)
  - TensorE: ctx[1, 1024] += e_chunk.T @ x_chunk accumulated in PSUM;
             denominator via matmul with a ones column
  - epilogue: reciprocal + scale + DMA out
"""

import sys

if "/opt/trn_rl_repo" not in sys.path:
    sys.path.insert(0, "/opt/trn_rl_repo")

import numpy as np

B, S, H = 32, 4096, 1024
NCORES = 8
BP = B // NCORES  # examples per core
P = 128  # SBUF partitions / rows per chunk
CH = 4  # chunks per supertile (DMA granularity = CH * 512KB)

TRACE = False
LAST_RESULT = None


def build_nc(
    bp=BP,
    s=S,
    h=H,
    ch=CH,
    mode="full",
    xbufs=5,
    dma_engines=("sync",),
    repeat=1,
    scrbufs=4,
    smallbufs=3,
    gp_mod=3,
    dma_alt=False,
    w2=False,
    wide=False,
    gp_sup=0,
    gp_reduce=False,
    dve_reduce=False,
    acc_reorder=False,
    scr_bf16=False,
    den_via_act=True,
):
    """mode: 'full' | 'dma' | 'dma_dve' | 'dma_dve_act' (probe variants).

    dma_engines: engine names to round-robin the x supertile loads across;
    the supertile's ch chunk-blocks are split evenly between them.
    repeat: run the whole body N times inside one NEFF (for timing: the
    per-exec dispatch floor in this environment is ~530us, so the true
    kernel time is measured from the slope over repeats).
    """
    import concourse.bacc as bacc
    import concourse.mybir as mybir
    from concourse import tile

    f32 = mybir.dt.float32
    nchunk = s // P
    nsup = nchunk // ch
    ncol = min(512, h)
    nde = len(dma_engines)
    assert ch % nde == 0

    def _is_gp(t, c):
        k = t * ch + c
        return bool(gp_mod) and (k % gp_mod == gp_mod - 1)

    nc = bacc.Bacc("TRN2", target_bir_lowering=False, debug=False)
    x = nc.declare_dram_parameter("x", [bp, s, h], f32, isOutput=False)
    w = nc.declare_dram_parameter("w", [1, h], f32, isOutput=False)
    out = nc.declare_dram_parameter("out", [bp, h], f32, isOutput=True)

    with tile.TileContext(nc) as tc:
        with (
            tc.tile_pool(name="const", bufs=1) as cpool,
            tc.tile_pool(name="xdata", bufs=xbufs) as xpool,
            tc.tile_pool(name="scratch", bufs=scrbufs) as scrpool,
            tc.tile_pool(name="small", bufs=smallbufs) as spool,
            tc.tile_pool(name="psum", bufs=2, space="PSUM") as ppool,
        ):
            w_bc = cpool.tile([P, h], f32)
            nc.sync.dma_start(w_bc[:], w[0:1, :].partition_broadcast(P))
            if w2:
                w_bc_g = cpool.tile([P, h], f32)
                nc.sync.dma_start(w_bc_g[:], w[0:1, :].partition_broadcast(P))
            else:
                w_bc_g = w_bc
            if wide:
                w_wide = cpool.tile([P, ch, h], f32)
                for c in range(ch):
                    nc.sync.dma_start(
                        w_wide[:, c, :], w[0:1, :].partition_broadcast(P)
                    )
            ones = cpool.tile([P, 1], f32)
            nc.vector.memset(ones[:], 1.0)
            tok = cpool.tile([P, 1], f32)

            for _rep in range(repeat):
              for b in range(bp):
                if mode == "full":
                    ctx_ps = ppool.tile([1, h], f32, tag="ctx")
                    if den_via_act:
                        den_all = spool.tile([P, nsup], f32, tag="den_all")
                    else:
                        den_ps = ppool.tile([ch, 1], f32, tag="den")
                for t in range(nsup):
                    xt = xpool.tile([P, ch, h], f32, tag="x")
                    src = x[b, t * ch * P : (t + 1) * ch * P, :].rearrange(
                        "(p c) h -> p c h", c=ch
                    )
                    if dma_alt:
                        deng = nc.sync if t % 2 == 0 else nc.scalar
                        deng.dma_start(xt[:], src)
                    else:
                        cpe = ch // nde
                        for d, ename in enumerate(dma_engines):
                            getattr(nc, ename).dma_start(
                                xt[:, d * cpe : (d + 1) * cpe, :],
                                src[:, d * cpe : (d + 1) * cpe, :],
                            )
                    if mode == "dma":
                        nc.scalar.activation(
                            tok[:],
                            xt[:, 0, 0:1],
                            mybir.ActivationFunctionType.Copy,
                        )
                        continue
                    en = spool.tile([P, ch], f32, tag="en")
                    if acc_reorder and not wide:
                        # emit DVE-chunk muls+accums before GP-chunk ones so
                        # ACT never head-of-line blocks on the slower gpsimd
                        order = [c for c in range(ch) if not _is_gp(t, c)] + [
                            c for c in range(ch) if _is_gp(t, c)
                        ]
                    else:
                        order = list(range(ch))
                    if wide:
                        wscr = scrpool.tile([P, ch, h], f32, tag="scr")
                        on_gp = gp_sup and (t % gp_sup == gp_sup - 1)
                        (nc.gpsimd if on_gp else nc.vector).tensor_tensor(
                            wscr[:], xt[:], w_wide[:], mybir.AluOpType.mult
                        )
                        if mode != "dma_dve":
                            for c in range(ch):
                                nc.scalar.activation(
                                    wscr[:, c, :],
                                    wscr[:, c, :],
                                    mybir.ActivationFunctionType.Copy,
                                    accum_out=en[:, c : c + 1],
                                )
                        scr = wscr[:, ch - 1, :]
                    else:
                      for c in order:
                        k = t * ch + c
                        scr = scrpool.tile(
                            [P, h], mybir.dt.bfloat16 if scr_bf16 else f32, tag="scr"
                        )
                        on_gp = _is_gp(t, c)
                        (nc.gpsimd if on_gp else nc.vector).tensor_tensor(
                            scr[:],
                            xt[:, c, :],
                            (w_bc_g if on_gp else w_bc)[:],
                            mybir.AluOpType.mult,
                        )
                        if mode == "dma_dve":
                            continue
                        if (on_gp and gp_reduce) or (not on_gp and dve_reduce):
                            nc.vector.tensor_reduce(
                                en[:, c : c + 1],
                                scr[:],
                                axis=mybir.AxisListType.X,
                                op=mybir.AluOpType.add,
                            )
                        else:
                            nc.scalar.activation(
                                scr[:],
                                scr[:],
                                mybir.ActivationFunctionType.Copy,
                                accum_out=en[:, c : c + 1],
                            )
                    if mode == "dma_dve":
                        nc.scalar.activation(
                            tok[:], scr[:, 0:1], mybir.ActivationFunctionType.Copy
                        )
                        continue
                    if mode == "dma_dve_act":
                        nc.vector.tensor_copy(tok[:], en[:, 0:1])
                        continue
                    th = spool.tile([P, ch], f32, tag="th")
                    nc.scalar.activation(
                        th[:], en[:], mybir.ActivationFunctionType.Tanh
                    )
                    e_t = spool.tile([P, ch], f32, tag="e_t")
                    if den_via_act:
                        nc.scalar.activation(
                            e_t[:],
                            th[:],
                            mybir.ActivationFunctionType.Exp,
                            accum_out=den_all[:, t : t + 1],
                        )
                    else:
                        nc.scalar.activation(
                            e_t[:], th[:], mybir.ActivationFunctionType.Exp
                        )
                        # denominator partials: den_ps[c] += sum_p e_t[p, c]
                        nc.tensor.matmul(
                            den_ps[0:ch, 0:1],
                            lhsT=e_t[:],
                            rhs=ones[:, 0:1],
                            start=(t == 0),
                            stop=(t == nsup - 1),
                        )
                    for c in range(ch):
                        k = t * ch + c
                        e_col = e_t[:, c : c + 1]
                        for n0 in range(0, h, ncol):
                            nc.tensor.matmul(
                                ctx_ps[0:1, n0 : n0 + ncol],
                                lhsT=e_col,
                                rhs=xt[:, c, n0 : n0 + ncol],
                                start=(k == 0),
                                stop=(k == nchunk - 1),
                            )
                if mode != "full":
                    continue
                # epilogue: denominator, reciprocal, scale, store
                den1 = ppool.tile([1, 1], f32, tag="den1")
                if den_via_act:
                    erows = spool.tile([P, 1], f32, tag="erows")
                    nc.vector.tensor_reduce(
                        erows[:],
                        den_all[:],
                        axis=mybir.AxisListType.X,
                        op=mybir.AluOpType.add,
                    )
                    nc.tensor.matmul(
                        den1[0:1, 0:1], lhsT=erows[:, 0:1], rhs=ones[:, 0:1]
                    )
                else:
                    dch = spool.tile([ch, 1], f32, tag="dch")
                    nc.scalar.activation(
                        dch[:], den_ps[0:ch, 0:1], mybir.ActivationFunctionType.Copy
                    )
                    nc.tensor.matmul(
                        den1[0:1, 0:1], lhsT=dch[0:ch, 0:1], rhs=ones[0:ch, 0:1]
                    )
                recip = spool.tile([1, 1], f32, tag="recip")
                nc.vector.reciprocal(recip[:], den1[0:1, 0:1])
                o = spool.tile([1, h], f32, tag="o")
                nc.vector.tensor_scalar_mul(o[:], ctx_ps[0:1, :], recip[0:1, 0:1])
                nc.sync.dma_start(out[b : b + 1, :], o[:])

            if mode != "full":
                o2 = cpool.tile([1, h], f32)
                nc.scalar.activation(
                    o2[:],
                    tok[0:1, 0:1].broadcast_to([1, h]),
                    mybir.ActivationFunctionType.Copy,
                )
                for b in range(bp):
                    nc.sync.dma_start(out[b : b + 1, :], o2[:])

    nc.finalize()
    return nc


_nc_cache = {}


def kernel(lstm_outputs, w_attn):
    global LAST_RESULT
    from concourse.bass_utils import run_bass_kernel_spmd

    key = "main"
    if key not in _nc_cache:
        _nc_cache[key] = build_nc()
    nc = _nc_cache[key]

    x = np.ascontiguousarray(np.asarray(lstm_outputs, dtype=np.float32))
    w = np.ascontiguousarray(np.asarray(w_attn, dtype=np.float32)).reshape(1, H)

    in_maps = [
        {"x": x[i * BP : (i + 1) * BP], "w": w} for i in range(NCORES)
    ]
    res = run_bass_kernel_spmd(
        nc, in_maps, core_ids=list(range(NCORES)), trace=TRACE
    )
    LAST_RESULT = res
    return np.concatenate([res.results[i]["out"] for i in range(NCORES)], axis=0)



# revision 18
# speedup vs baseline: 1.3682x; 1.3682x over previous
"""Attention pooling kernel for Trainium2 (8 NeuronCores, data-parallel).

Computes, per example b:
    energy[s] = tanh(dot(x[b, s, :], w))
    attn      = softmax(energy) over s
    out[b, h] = sum_s attn[s] * x[b, s, h]

tanh bounds energy to [-1, 1], so exp() needs no max-subtraction: we
accumulate the unnormalized weighted sum and the denominator in one pass
over the data (single HBM read of x — the memory roofline).

Per-core mapping (shard = B/8 = 4 examples):
  - DMA: stream x in supertiles [128 rows, CH*1024]; rows are mapped
    p-major (s = p*CH + c) so each partition reads one contiguous
    CH*4KB run from DRAM. The softmax sums are permutation-invariant
    across rows, so any row->$(p,c) mapping works.
  - VectorE: elementwise x*w product
  - ScalarE: Copy-activation with accum_out (free-dim sum) -> energy per row,
    then tanh and exp (the fused DVE reduce ops fail on this runtime)
  - TensorE: ctx[1, 1024] += e_chunk.T @ x_chunk accumulated in PSUM;
             denominator via matmul with a ones column
  - epilogue: reciprocal + scale + DMA out
"""

import sys

if "/opt/trn_rl_repo" not in sys.path:
    sys.path.insert(0, "/opt/trn_rl_repo")

import numpy as np

B, S, H = 32, 4096, 1024
NCORES = 8
BP = B // NCORES  # examples per core
P = 128  # SBUF partitions / rows per chunk
CH = 4  # chunks per supertile (DMA granularity = CH * 512KB)

TRACE = False
LAST_RESULT = None


def build_nc(
    bp=BP,
    s=S,
    h=H,
    ch=8,
    mode="full",
    xbufs=3,
    dma_engines=("sync", "sync", "sync", "scalar"),
    repeat=1,
    scrbufs=None,
    smallbufs=3,
    gp_mod=0,
    dma_alt=False,
    w2=False,
    wide=False,
    gp_sup=0,
    gp_reduce=False,
    dve_reduce=False,
    acc_reorder=False,
    scr_bf16=False,
    den_via_act=True,
    f32r=True,
    wfold=True,
    er_via_dve=True,
    red_mod=4,
):
    """mode: 'full' | 'dma' | 'dma_dve' | 'dma_dve_act' (probe variants).

    dma_engines: engine names to round-robin the x supertile loads across;
    the supertile's ch chunk-blocks are split evenly between them.
    repeat: run the whole body N times inside one NEFF (for timing: the
    per-exec dispatch floor in this environment is ~530us, so the true
    kernel time is measured from the slope over repeats).
    """
    import concourse.bacc as bacc
    import concourse.mybir as mybir
    from concourse import tile

    f32 = mybir.dt.float32
    f32r = mybir.dt.float32r if f32r else None
    assert not (wfold and f32r is None), "wfold requires f32r"
    scr_dt = f32r if (wfold and f32r) else (mybir.dt.bfloat16 if scr_bf16 else f32)
    if scrbufs is None:
        scrbufs = 3 if wide else (2 * ch if wfold else 4)
    nchunk = s // P
    nsup = nchunk // ch
    ncol = min(512, h)
    nde = len(dma_engines)
    assert ch % nde == 0

    def _is_gp(t, c):
        k = t * ch + c
        return bool(gp_mod) and (k % gp_mod == gp_mod - 1)

    nc = bacc.Bacc("TRN2", target_bir_lowering=False, debug=False)
    x = nc.declare_dram_parameter("x", [bp, s, h], f32, isOutput=False)
    w = nc.declare_dram_parameter("w", [1, h], f32, isOutput=False)
    out = nc.declare_dram_parameter("out", [bp, h], f32, isOutput=True)

    with tile.TileContext(nc) as tc:
        with (
            tc.tile_pool(name="const", bufs=1) as cpool,
            tc.tile_pool(name="xdata", bufs=xbufs) as xpool,
            tc.tile_pool(name="scratch", bufs=scrbufs) as scrpool,
            tc.tile_pool(name="small", bufs=smallbufs) as spool,
            tc.tile_pool(name="psum", bufs=2, space="PSUM") as ppool,
        ):
            w_bc = cpool.tile([P, h], f32)
            nc.sync.dma_start(w_bc[:], w[0:1, :].partition_broadcast(P))
            if w2:
                w_bc_g = cpool.tile([P, h], f32)
                nc.sync.dma_start(w_bc_g[:], w[0:1, :].partition_broadcast(P))
            else:
                w_bc_g = w_bc
            if wide:
                w_wide = cpool.tile([P, ch, h], f32)
                for c in range(ch):
                    nc.sync.dma_start(
                        w_wide[:, c, :], w[0:1, :].partition_broadcast(P)
                    )
            ones = cpool.tile([P, 1], f32)
            nc.vector.memset(ones[:], 1.0)
            tok = cpool.tile([P, 1], f32)
            if wfold:
                winv = cpool.tile([1, h], f32)
                nc.vector.reciprocal(winv[:], w_bc[0:1, :])
                garbage = cpool.tile([P, h], f32)

            for _rep in range(repeat):
              for b in range(bp):
                if mode == "full":
                    ctx_ps = ppool.tile([1, h], f32, tag="ctx")
                    if den_via_act:
                        den_all = spool.tile([P, nsup], f32, tag="den_all")
                    else:
                        den_ps = ppool.tile([ch, 1], f32, tag="den")
                for t in range(nsup):
                    xt = xpool.tile([P, ch, h], f32, tag="x")
                    src = x[b, t * ch * P : (t + 1) * ch * P, :].rearrange(
                        "(p c) h -> p c h", c=ch
                    )
                    if dma_alt:
                        deng = nc.sync if t % 2 == 0 else nc.scalar
                        deng.dma_start(xt[:], src)
                    else:
                        cpe = ch // nde
                        for d, ename in enumerate(dma_engines):
                            getattr(nc, ename).dma_start(
                                xt[:, d * cpe : (d + 1) * cpe, :],
                                src[:, d * cpe : (d + 1) * cpe, :],
                            )
                    if mode == "dma":
                        nc.scalar.activation(
                            tok[:],
                            xt[:, 0, 0:1],
                            mybir.ActivationFunctionType.Copy,
                        )
                        continue
                    en = spool.tile([P, ch], f32, tag="en")
                    if acc_reorder and not wide:
                        # emit DVE-chunk muls+accums before GP-chunk ones so
                        # ACT never head-of-line blocks on the slower gpsimd
                        order = [c for c in range(ch) if not _is_gp(t, c)] + [
                            c for c in range(ch) if _is_gp(t, c)
                        ]
                    else:
                        order = list(range(ch))
                    if wide:
                        scr_by_c = {}
                        wscr = scrpool.tile([P, ch, h], scr_dt, tag="scr")
                        on_gp = gp_sup and (t % gp_sup == gp_sup - 1)
                        (nc.gpsimd if on_gp else nc.vector).tensor_tensor(
                            wscr[:], xt[:], w_wide[:], mybir.AluOpType.mult
                        )
                        if mode != "dma_dve":
                            for c in range(ch):
                                k = t * ch + c
                                acc_in = (
                                    wscr[:, c, :].bitcast(f32)
                                    if wfold
                                    else wscr[:, c, :]
                                )
                                acc_out = garbage[:] if wfold else wscr[:, c, :]
                                if red_mod and k % red_mod == red_mod - 1:
                                    nc.vector.tensor_reduce(
                                        en[:, c : c + 1],
                                        acc_in,
                                        axis=mybir.AxisListType.X,
                                        op=mybir.AluOpType.add,
                                    )
                                else:
                                    nc.scalar.activation(
                                        acc_out,
                                        acc_in,
                                        mybir.ActivationFunctionType.Copy,
                                        accum_out=en[:, c : c + 1],
                                    )
                        for c in range(ch):
                            scr_by_c[c] = wscr[:, c, :]
                        scr = wscr[:, ch - 1, :]
                    else:
                      scr_by_c = {}
                      for c in order:
                        k = t * ch + c
                        scr = scrpool.tile([P, h], scr_dt, tag="scr")
                        scr_by_c[c] = scr
                        on_gp = _is_gp(t, c)
                        (nc.gpsimd if on_gp else nc.vector).tensor_tensor(
                            scr[:],
                            xt[:, c, :],
                            (w_bc_g if on_gp else w_bc)[:],
                            mybir.AluOpType.mult,
                        )
                        if mode == "dma_dve":
                            continue
                        acc_in = scr[:].bitcast(f32) if wfold else scr[:]
                        acc_out = garbage[:] if wfold else scr[:]
                        if (
                            (on_gp and gp_reduce)
                            or (not on_gp and dve_reduce)
                            or (red_mod and k % red_mod == red_mod - 1)
                        ):
                            nc.vector.tensor_reduce(
                                en[:, c : c + 1],
                                acc_in,
                                axis=mybir.AxisListType.X,
                                op=mybir.AluOpType.add,
                            )
                        else:
                            nc.scalar.activation(
                                acc_out,
                                acc_in,
                                mybir.ActivationFunctionType.Copy,
                                accum_out=en[:, c : c + 1],
                            )
                    if mode == "dma_dve":
                        tok_src = scr[:, 0:1].bitcast(f32) if wfold else scr[:, 0:1]
                        nc.scalar.activation(
                            tok[:], tok_src, mybir.ActivationFunctionType.Copy
                        )
                        continue
                    if mode == "dma_dve_act":
                        nc.vector.tensor_copy(tok[:], en[:, 0:1])
                        continue
                    th = spool.tile([P, ch], f32, tag="th")
                    nc.scalar.activation(
                        th[:], en[:], mybir.ActivationFunctionType.Tanh
                    )
                    e_t = spool.tile([P, ch], f32, tag="e_t")
                    if den_via_act:
                        nc.scalar.activation(
                            e_t[:],
                            th[:],
                            mybir.ActivationFunctionType.Exp,
                            accum_out=den_all[:, t : t + 1],
                        )
                    else:
                        nc.scalar.activation(
                            e_t[:], th[:], mybir.ActivationFunctionType.Exp
                        )
                        # denominator partials: den_ps[c] += sum_p e_t[p, c]
                        nc.tensor.matmul(
                            den_ps[0:ch, 0:1],
                            lhsT=e_t[:],
                            rhs=ones[:, 0:1],
                            start=(t == 0),
                            stop=(t == nsup - 1),
                        )
                    if wfold and er_via_dve:
                        e_r = spool.tile([P, ch], f32r, tag="e_r")
                        nc.vector.tensor_copy(e_r[:], e_t[:])
                        e_src = e_r
                    else:
                        e_src = e_t
                    for c in range(ch):
                        k = t * ch + c
                        e_col = e_src[:, c : c + 1]
                        if f32r is not None and e_col.dtype != f32r:
                            e_col = e_col.bitcast(f32r)
                        for n0 in range(0, h, ncol):
                            if wfold:
                                rhs = scr_by_c[c][:, n0 : n0 + ncol]
                            else:
                                rhs = xt[:, c, n0 : n0 + ncol]
                                if f32r is not None:
                                    rhs = rhs.bitcast(f32r)
                            nc.tensor.matmul(
                                ctx_ps[0:1, n0 : n0 + ncol],
                                lhsT=e_col,
                                rhs=rhs,
                                start=(k == 0),
                                stop=(k == nchunk - 1),
                            )
                if mode != "full":
                    continue
                # epilogue: denominator, reciprocal, scale, store
                den1 = ppool.tile([1, 1], f32, tag="den1")
                if den_via_act:
                    erows = spool.tile([P, 1], f32, tag="erows")
                    nc.vector.tensor_reduce(
                        erows[:],
                        den_all[:],
                        axis=mybir.AxisListType.X,
                        op=mybir.AluOpType.add,
                    )
                    nc.tensor.matmul(
                        den1[0:1, 0:1], lhsT=erows[:, 0:1], rhs=ones[:, 0:1]
                    )
                else:
                    dch = spool.tile([ch, 1], f32, tag="dch")
                    nc.scalar.activation(
                        dch[:], den_ps[0:ch, 0:1], mybir.ActivationFunctionType.Copy
                    )
                    nc.tensor.matmul(
                        den1[0:1, 0:1], lhsT=dch[0:ch, 0:1], rhs=ones[0:ch, 0:1]
                    )
                recip = spool.tile([1, 1], f32, tag="recip")
                nc.vector.reciprocal(recip[:], den1[0:1, 0:1])
                o = spool.tile([1, h], f32, tag="o")
                if wfold:
                    nc.vector.scalar_tensor_tensor(
                        o[:],
                        ctx_ps[0:1, :],
                        recip[0:1, 0:1],
                        winv[:],
                        mybir.AluOpType.mult,
                        mybir.AluOpType.mult,
                    )
                else:
                    nc.vector.tensor_scalar_mul(o[:], ctx_ps[0:1, :], recip[0:1, 0:1])
                nc.sync.dma_start(out[b : b + 1, :], o[:])

            if mode != "full":
                o2 = cpool.tile([1, h], f32)
                nc.scalar.activation(
                    o2[:],
                    tok[0:1, 0:1].broadcast_to([1, h]),
                    mybir.ActivationFunctionType.Copy,
                )
                for b in range(bp):
                    nc.sync.dma_start(out[b : b + 1, :], o2[:])

    nc.finalize()
    return nc


_nc_cache = {}


def kernel(lstm_outputs, w_attn):
    global LAST_RESULT
    from concourse.bass_utils import run_bass_kernel_spmd

    key = "main"
    if key not in _nc_cache:
        _nc_cache[key] = build_nc()
    nc = _nc_cache[key]

    x = np.ascontiguousarray(np.asarray(lstm_outputs, dtype=np.float32))
    w = np.ascontiguousarray(np.asarray(w_attn, dtype=np.float32)).reshape(1, H)

    in_maps = [
        {"x": x[i * BP : (i + 1) * BP], "w": w} for i in range(NCORES)
    ]
    res = run_bass_kernel_spmd(
        nc, in_maps, core_ids=list(range(NCORES)), trace=TRACE
    )
    LAST_RESULT = res
    return np.concatenate([res.results[i]["out"] for i in range(NCORES)], axis=0)



# revision 26
# speedup vs baseline: 1.3797x; 1.0084x over previous
"""Attention pooling kernel for Trainium2 (8 NeuronCores, data-parallel).

Computes, per example b:
    energy[s] = tanh(dot(x[b, s, :], w))
    attn      = softmax(energy) over s
    out[b, h] = sum_s attn[s] * x[b, s, h]

tanh bounds energy to [-1, 1], so exp() needs no max-subtraction: we
accumulate the unnormalized weighted sum and the denominator in one pass
over the data (single HBM read of x — the memory roofline, ~188us/core
for 64MB at the ~2.7TB/s 8-core-aggregate HBM rate this box sustains).

Per-core mapping (shard = B/8 = 4 examples), measured ~193us vs the
~188-190us pure-DMA floor:
  - DMA: stream x in supertiles [128 rows, ch*4KB]; rows are mapped
    p-major (s = p*ch + c) so each partition reads one contiguous
    ch*4KB run from DRAM. 3:1 split across the two HWDGE rings
    (sync:scalar) measures fastest; the softmax sums are permutation-
    invariant across rows, so any row->(p,c) mapping works.
  - VectorE: scr = x*w product, written as float32r (the rounding
    producer the PE's fast-fp32 mode requires). GPSIMD must stay idle:
    its SBUF port is an exclusive lock shared with DVE, so offloading
    work to it stalls DVE (+23us).
  - ScalarE: Copy-activation with accum_out (free-dim sum of scr) ->
    energy per row for 3 of 4 chunks, then tanh and exp (+ den via
    exp's accum_out). Every 4th chunk reduces on DVE instead
    (red_mod=4) — keeps ACT off the supertile critical path.
  - TensorE: ctx[1, 1024] += e_chunk.T @ scr_chunk in PSUM, all float32r
    (1 cyc/row vs plain fp32's 4): w folded into the sum, divided back
    out in the epilogue (ctx/w has the same relative error as ctx).
  - epilogue: den = ones.T @ rowsums, reciprocal, o = ctx*recip*winv in
    one scalar_tensor_tensor, DMA out.
"""

import sys

if "/opt/trn_rl_repo" not in sys.path:
    sys.path.insert(0, "/opt/trn_rl_repo")

import numpy as np

B, S, H = 32, 4096, 1024
NCORES = 8
BP = B // NCORES  # examples per core
P = 128  # SBUF partitions / rows per chunk
CH = 4  # chunks per supertile (DMA granularity = CH * 512KB)

TRACE = False
LAST_RESULT = None


def build_nc(
    bp=BP,
    s=S,
    h=H,
    ch=8,
    mode="full",
    xbufs=3,
    dma_engines=("sync", "sync", "sync", "scalar"),
    repeat=1,
    scrbufs=None,
    smallbufs=3,
    gp_mod=0,
    dma_alt=False,
    w2=False,
    wide=False,
    gp_sup=0,
    gp_reduce=False,
    dve_reduce=False,
    acc_reorder=False,
    scr_bf16=False,
    den_via_act=True,
    f32r=True,
    wfold=True,
    er_via_dve=True,
    red_mod=4,
    red_last=0,
    nl_split=1,
    pbufs=2,
):
    """mode: 'full' | 'dma' | 'dma_dve' | 'dma_dve_act' (probe variants).

    dma_engines: engine names to round-robin the x supertile loads across;
    the supertile's ch chunk-blocks are split evenly between them.
    repeat: run the whole body N times inside one NEFF (for timing: the
    per-exec dispatch floor in this environment is ~530us, so the true
    kernel time is measured from the slope over repeats).
    """
    import concourse.bacc as bacc
    import concourse.mybir as mybir
    from concourse import tile

    f32 = mybir.dt.float32
    f32r = mybir.dt.float32r if f32r else None
    assert not (wfold and f32r is None), "wfold requires f32r"
    scr_dt = f32r if (wfold and f32r) else (mybir.dt.bfloat16 if scr_bf16 else f32)
    if scrbufs is None:
        scrbufs = 3 if wide else (2 * ch if wfold else 4)
    nchunk = s // P
    nsup = nchunk // ch
    ncol = min(512, h)
    nde = len(dma_engines)
    assert ch % nde == 0

    def _is_gp(t, c):
        k = t * ch + c
        return bool(gp_mod) and (k % gp_mod == gp_mod - 1)

    nc = bacc.Bacc("TRN2", target_bir_lowering=False, debug=False)
    x = nc.declare_dram_parameter("x", [bp, s, h], f32, isOutput=False)
    w = nc.declare_dram_parameter("w", [1, h], f32, isOutput=False)
    out = nc.declare_dram_parameter("out", [bp, h], f32, isOutput=True)

    with tile.TileContext(nc) as tc:
        with (
            tc.tile_pool(name="const", bufs=1) as cpool,
            tc.tile_pool(name="xdata", bufs=xbufs) as xpool,
            tc.tile_pool(name="scratch", bufs=scrbufs) as scrpool,
            tc.tile_pool(name="small", bufs=smallbufs) as spool,
            tc.tile_pool(name="psum", bufs=pbufs, space="PSUM") as ppool,
        ):
            w_bc = cpool.tile([P, h], f32)
            nc.sync.dma_start(w_bc[:], w[0:1, :].partition_broadcast(P))
            if w2:
                w_bc_g = cpool.tile([P, h], f32)
                nc.sync.dma_start(w_bc_g[:], w[0:1, :].partition_broadcast(P))
            else:
                w_bc_g = w_bc
            if wide:
                w_wide = cpool.tile([P, ch, h], f32)
                for c in range(ch):
                    nc.sync.dma_start(
                        w_wide[:, c, :], w[0:1, :].partition_broadcast(P)
                    )
            ones = cpool.tile([P, 1], f32)
            nc.vector.memset(ones[:], 1.0)
            tok = cpool.tile([P, 1], f32)
            if wfold:
                winv = cpool.tile([1, h], f32)
                nc.vector.reciprocal(winv[:], w_bc[0:1, :])
                garbage = cpool.tile([P, h], f32)

            for _rep in range(repeat):
              for b in range(bp):
                if mode == "full":
                    ctx_ps = ppool.tile([1, h], f32, tag="ctx")
                    if den_via_act:
                        den_all = spool.tile([P, nsup * nl_split], f32, tag="den_all")
                    else:
                        assert nl_split == 1
                        den_ps = ppool.tile([ch, 1], f32, tag="den")
                for t in range(nsup):
                    xt = xpool.tile([P, ch, h], f32, tag="x")
                    src = x[b, t * ch * P : (t + 1) * ch * P, :].rearrange(
                        "(p c) h -> p c h", c=ch
                    )
                    if dma_alt:
                        deng = nc.sync if t % 2 == 0 else nc.scalar
                        deng.dma_start(xt[:], src)
                    else:
                        cpe = ch // nde
                        for d, ename in enumerate(dma_engines):
                            getattr(nc, ename).dma_start(
                                xt[:, d * cpe : (d + 1) * cpe, :],
                                src[:, d * cpe : (d + 1) * cpe, :],
                            )
                    if mode == "dma":
                        nc.scalar.activation(
                            tok[:],
                            xt[:, 0, 0:1],
                            mybir.ActivationFunctionType.Copy,
                        )
                        continue
                    en = spool.tile([P, ch], f32, tag="en")
                    if acc_reorder and not wide:
                        # emit DVE-chunk muls+accums before GP-chunk ones so
                        # ACT never head-of-line blocks on the slower gpsimd
                        order = [c for c in range(ch) if not _is_gp(t, c)] + [
                            c for c in range(ch) if _is_gp(t, c)
                        ]
                    else:
                        order = list(range(ch))
                    if wide:
                        scr_by_c = {}
                        wscr = scrpool.tile([P, ch, h], scr_dt, tag="scr")
                        on_gp = gp_sup and (t % gp_sup == gp_sup - 1)
                        (nc.gpsimd if on_gp else nc.vector).tensor_tensor(
                            wscr[:], xt[:], w_wide[:], mybir.AluOpType.mult
                        )
                        if mode != "dma_dve":
                            for c in range(ch):
                                k = t * ch + c
                                acc_in = (
                                    wscr[:, c, :].bitcast(f32)
                                    if wfold
                                    else wscr[:, c, :]
                                )
                                acc_out = garbage[:] if wfold else wscr[:, c, :]
                                if (red_mod and k % red_mod == red_mod - 1) or (
                                    red_last and c >= ch - red_last
                                ):
                                    nc.vector.tensor_reduce(
                                        en[:, c : c + 1],
                                        acc_in,
                                        axis=mybir.AxisListType.X,
                                        op=mybir.AluOpType.add,
                                    )
                                else:
                                    nc.scalar.activation(
                                        acc_out,
                                        acc_in,
                                        mybir.ActivationFunctionType.Copy,
                                        accum_out=en[:, c : c + 1],
                                    )
                        for c in range(ch):
                            scr_by_c[c] = wscr[:, c, :]
                        scr = wscr[:, ch - 1, :]
                    else:
                      scr_by_c = {}
                      for c in order:
                        k = t * ch + c
                        scr = scrpool.tile([P, h], scr_dt, tag="scr")
                        scr_by_c[c] = scr
                        on_gp = _is_gp(t, c)
                        (nc.gpsimd if on_gp else nc.vector).tensor_tensor(
                            scr[:],
                            xt[:, c, :],
                            (w_bc_g if on_gp else w_bc)[:],
                            mybir.AluOpType.mult,
                        )
                        if mode == "dma_dve":
                            continue
                        acc_in = scr[:].bitcast(f32) if wfold else scr[:]
                        acc_out = garbage[:] if wfold else scr[:]
                        if (
                            (on_gp and gp_reduce)
                            or (not on_gp and dve_reduce)
                            or (red_mod and k % red_mod == red_mod - 1)
                            or (red_last and c >= ch - red_last)
                        ):
                            nc.vector.tensor_reduce(
                                en[:, c : c + 1],
                                acc_in,
                                axis=mybir.AxisListType.X,
                                op=mybir.AluOpType.add,
                            )
                        else:
                            nc.scalar.activation(
                                acc_out,
                                acc_in,
                                mybir.ActivationFunctionType.Copy,
                                accum_out=en[:, c : c + 1],
                            )
                    if mode == "dma_dve":
                        tok_src = scr[:, 0:1].bitcast(f32) if wfold else scr[:, 0:1]
                        nc.scalar.activation(
                            tok[:], tok_src, mybir.ActivationFunctionType.Copy
                        )
                        continue
                    if mode == "dma_dve_act":
                        nc.vector.tensor_copy(tok[:], en[:, 0:1])
                        continue
                    gs = ch // nl_split
                    for gi in range(nl_split):
                      g0 = gi * gs
                      gsl = slice(g0, g0 + gs)
                      th = spool.tile([P, gs], f32, tag="th")
                      nc.scalar.activation(
                          th[:], en[:, gsl], mybir.ActivationFunctionType.Tanh
                      )
                      e_t = spool.tile([P, gs], f32, tag="e_t")
                      if den_via_act:
                        dcol = t * nl_split + gi
                        nc.scalar.activation(
                            e_t[:],
                            th[:],
                            mybir.ActivationFunctionType.Exp,
                            accum_out=den_all[:, dcol : dcol + 1],
                        )
                      else:
                        nc.scalar.activation(
                            e_t[:], th[:], mybir.ActivationFunctionType.Exp
                        )
                        # denominator partials: den_ps[c] += sum_p e_t[p, c]
                        nc.tensor.matmul(
                            den_ps[0:ch, 0:1],
                            lhsT=e_t[:],
                            rhs=ones[:, 0:1],
                            start=(t == 0),
                            stop=(t == nsup - 1),
                        )
                      if wfold and er_via_dve:
                        e_r = spool.tile([P, gs], f32r, tag="e_r")
                        nc.vector.tensor_copy(e_r[:], e_t[:])
                        e_src = e_r
                      else:
                        e_src = e_t
                      for c in range(g0, g0 + gs):
                        k = t * ch + c
                        e_col = e_src[:, c - g0 : c - g0 + 1]
                        if f32r is not None and e_col.dtype != f32r:
                            e_col = e_col.bitcast(f32r)
                        for n0 in range(0, h, ncol):
                            if wfold:
                                rhs = scr_by_c[c][:, n0 : n0 + ncol]
                            else:
                                rhs = xt[:, c, n0 : n0 + ncol]
                                if f32r is not None:
                                    rhs = rhs.bitcast(f32r)
                            nc.tensor.matmul(
                                ctx_ps[0:1, n0 : n0 + ncol],
                                lhsT=e_col,
                                rhs=rhs,
                                start=(k == 0),
                                stop=(k == nchunk - 1),
                            )
                if mode != "full":
                    continue
                # epilogue: denominator, reciprocal, scale, store
                den1 = ppool.tile([1, 1], f32, tag="den1")
                if den_via_act:
                    erows = spool.tile([P, 1], f32, tag="erows")
                    nc.vector.tensor_reduce(
                        erows[:],
                        den_all[:],
                        axis=mybir.AxisListType.X,
                        op=mybir.AluOpType.add,
                    )
                    nc.tensor.matmul(
                        den1[0:1, 0:1], lhsT=erows[:, 0:1], rhs=ones[:, 0:1]
                    )
                else:
                    dch = spool.tile([ch, 1], f32, tag="dch")
                    nc.scalar.activation(
                        dch[:], den_ps[0:ch, 0:1], mybir.ActivationFunctionType.Copy
                    )
                    nc.tensor.matmul(
                        den1[0:1, 0:1], lhsT=dch[0:ch, 0:1], rhs=ones[0:ch, 0:1]
                    )
                recip = spool.tile([1, 1], f32, tag="recip")
                nc.vector.reciprocal(recip[:], den1[0:1, 0:1])
                o = spool.tile([1, h], f32, tag="o")
                if wfold:
                    nc.vector.scalar_tensor_tensor(
                        o[:],
                        ctx_ps[0:1, :],
                        recip[0:1, 0:1],
                        winv[:],
                        mybir.AluOpType.mult,
                        mybir.AluOpType.mult,
                    )
                else:
                    nc.vector.tensor_scalar_mul(o[:], ctx_ps[0:1, :], recip[0:1, 0:1])
                nc.sync.dma_start(out[b : b + 1, :], o[:])

            if mode != "full":
                o2 = cpool.tile([1, h], f32)
                nc.scalar.activation(
                    o2[:],
                    tok[0:1, 0:1].broadcast_to([1, h]),
                    mybir.ActivationFunctionType.Copy,
                )
                for b in range(bp):
                    nc.sync.dma_start(out[b : b + 1, :], o2[:])

    nc.finalize()
    return nc


_nc_cache = {}


def kernel(lstm_outputs, w_attn):
    global LAST_RESULT
    from concourse.bass_utils import run_bass_kernel_spmd

    key = "main"
    if key not in _nc_cache:
        _nc_cache[key] = build_nc()
    nc = _nc_cache[key]

    x = np.ascontiguousarray(np.asarray(lstm_outputs, dtype=np.float32))
    w = np.ascontiguousarray(np.asarray(w_attn, dtype=np.float32)).reshape(1, H)

    in_maps = [
        {"x": x[i * BP : (i + 1) * BP], "w": w} for i in range(NCORES)
    ]
    res = run_bass_kernel_spmd(
        nc, in_maps, core_ids=list(range(NCORES)), trace=TRACE
    )
    LAST_RESULT = res
    return np.concatenate([res.results[i]["out"] for i in range(NCORES)], axis=0)

